# revision 13
# baseline (speedup 1.0000x reference)
"""GAT regression kernel for Trainium2, distributed over 8 NeuronCores.

Strategy (dst-node sharding, edge lists sorted by destination):
  - Each core owns a contiguous range of destination nodes (N/8).
  - Host sorts edges (incl. self-loops) by dst, shards them by dst range,
    groups them into 128-node dst tiles, and packs them into fixed-size
    chunk slots (128 edges per chunk, C_FIX chunks per tile).
  - Per layer: each core computes the node transform h_aug = x @ W_aug for
    its own nodes ([h | 0 | s_src | s_dst] + a const-one column), then an
    AllGather replicates the full bf16 node table to every core.
  - Edge phase: batched indirect-DMA gathers pull h_aug rows by src id and
    s_dst by dst id; attention weights w = exp(max(z, 0.2 z)) are computed
    in bulk; a per-chunk "scaled one-hot" (w_e at column dst_e) feeds a
    PE matmul that segment-sums w_e * [h_src | 1] into PSUM per dst tile.
  - Layer-2 outputs are column-summed via ones-matmuls, AllReduced, and the
    final linear head produces the [1,1] output on every core.
"""

import sys

for _p in ("/opt/trn_rl_repo", "/opt/trn_rl_repo/concourse"):
    if _p not in sys.path:
        sys.path.insert(0, _p)

import numpy as np
import ml_dtypes

import concourse.bass as bass
import concourse.mybir as mybir
import concourse.tile as tile
from concourse import bacc

P = 128
D = 64            # feature dim (both layers)
DT = 68           # table row: [h(0:64), one(64), s_src(65), s_dst(66), pad]
CORES = 8
NEG = 0.2         # leaky relu slope
EPS = 1e-16

# problem sizes (hardcoded per spec)
N_NODES = 100000
E_RAW = 1200000

f32 = mybir.dt.float32
bf16 = mybir.dt.bfloat16
i32 = mybir.dt.int32
bf16_np = ml_dtypes.bfloat16


class Cfg:
    def __init__(self, n_nodes):
        assert n_nodes % CORES == 0
        self.n_nodes = n_nodes
        self.nown = n_nodes // CORES          # real nodes per core
        self.nt = -(-self.nown // P)          # dst tiles per core
        self.npad = self.nt * P               # padded nodes per core
        self.ntab = CORES * self.npad         # padded global table rows
        self.last_rows = self.nown - (self.nt - 1) * P


def _pad_gid(node_ids, cfg):
    """global node id -> row in the padded AllGather table"""
    return (node_ids // cfg.nown) * cfg.npad + (node_ids % cfg.nown)


def prep_inputs(inputs, cfg):
    """Host-side sharding: index manipulation + dtype/layout prep only."""
    x = np.asarray(inputs["x"], np.float32)
    ei = np.asarray(inputs["edge_index"], np.int64)
    loop = np.arange(cfg.n_nodes, dtype=np.int64)
    src = np.concatenate([ei[0], loop])
    dst = np.concatenate([ei[1], loop])
    order = np.argsort(dst, kind="stable")
    src_s, dst_s = src[order], dst[order]

    core_bounds = np.searchsorted(dst_s, np.arange(CORES + 1) * cfg.nown)
    # first pass: global max chunks per tile
    c_fix = 1
    per_core = []
    for k in range(CORES):
        sl = slice(core_bounds[k], core_bounds[k + 1])
        dloc = dst_s[sl] - k * cfg.nown
        sglo = src_s[sl]
        tb = np.searchsorted(dloc, np.arange(cfg.nt + 1) * P)
        cnts = np.diff(tb)
        c_fix = max(c_fix, int(-(-cnts.max() // P)))
        per_core.append((dloc, sglo, tb))

    ctot = cfg.nt * c_fix
    in_maps = []
    w1 = np.asarray(inputs["W1"], np.float32)
    w2 = np.asarray(inputs["W2"], np.float32)
    shared = {
        "w1": w1,
        "w1t": np.ascontiguousarray(w1.T),
        "as1": np.asarray(inputs["a_src1"], np.float32).reshape(D, 1),
        "ad1": np.asarray(inputs["a_dst1"], np.float32).reshape(D, 1),
        "w2": w2,
        "w2t": np.ascontiguousarray(w2.T),
        "as2": np.asarray(inputs["a_src2"], np.float32).reshape(D, 1),
        "ad2": np.asarray(inputs["a_dst2"], np.float32).reshape(D, 1),
        "b1t": np.ascontiguousarray(
            np.broadcast_to(np.asarray(inputs["b1"], np.float32), (P, D))),
        "b2t": np.ascontiguousarray(
            np.broadcast_to(np.asarray(inputs["b2"], np.float32), (P, D))),
        "woutt": np.asarray(inputs["W_out"], np.float32).reshape(1, D),
        "bout": np.asarray(inputs["b_out"], np.float32).reshape(1, 1),
    }
    for k in range(CORES):
        dloc, sglo, tb = per_core[k]
        src_arr = np.zeros((P, ctot), np.int32)
        rel_arr = np.full((P, ctot), -1, np.int32)
        abs_arr = np.zeros((P, ctot), np.int32)
        for t in range(cfg.nt):
            e0, e1 = tb[t], tb[t + 1]
            n = e1 - e0
            if n == 0:
                continue
            nslots = c_fix * P
            s_pad = np.zeros(nslots, np.int64)
            r_pad = np.full(nslots, -1, np.int64)
            a_pad = np.zeros(nslots, np.int64)
            s_pad[:n] = _pad_gid(sglo[e0:e1], cfg)
            r_pad[:n] = dloc[e0:e1] - t * P
            a_pad[:n] = _pad_gid(dloc[e0:e1] + k * cfg.nown, cfg)
            cs = slice(t * c_fix, (t + 1) * c_fix)
            src_arr[:, cs] = s_pad.reshape(c_fix, P).T
            rel_arr[:, cs] = r_pad.reshape(c_fix, P).T
            abs_arr[:, cs] = a_pad.reshape(c_fix, P).T
        xt = np.zeros((D, cfg.npad), np.float32)
        xt[:, :cfg.nown] = x[k * cfg.nown:(k + 1) * cfg.nown].T
        in_maps.append({
            "xt": xt, "esrc": src_arr, "erel": rel_arr, "eabs": abs_arr,
            **shared,
        })
    return in_maps, c_fix


def build_kernel(cfg, c_fix, tb_n=8, tb_e=8, debug_dumps=False):
    """Build the SPMD Bass program (same program for all 8 cores)."""
    ctot = cfg.nt * c_fix
    nc = bacc.Bacc("TRN2", target_bir_lowering=False, debug=False,
                   num_devices=CORES)

    # I/O
    xt_d = nc.dram_tensor("xt", [D, cfg.npad], f32, kind="ExternalInput")
    esrc_d = nc.dram_tensor("esrc", [P, ctot], i32, kind="ExternalInput")
    erel_d = nc.dram_tensor("erel", [P, ctot], i32, kind="ExternalInput")
    eabs_d = nc.dram_tensor("eabs", [P, ctot], i32, kind="ExternalInput")
    w1_d = nc.dram_tensor("w1", [D, D], f32, kind="ExternalInput")
    w1t_d = nc.dram_tensor("w1t", [D, D], f32, kind="ExternalInput")
    as1_d = nc.dram_tensor("as1", [D, 1], f32, kind="ExternalInput")
    ad1_d = nc.dram_tensor("ad1", [D, 1], f32, kind="ExternalInput")
    w2_d = nc.dram_tensor("w2", [D, D], f32, kind="ExternalInput")
    w2t_d = nc.dram_tensor("w2t", [D, D], f32, kind="ExternalInput")
    as2_d = nc.dram_tensor("as2", [D, 1], f32, kind="ExternalInput")
    ad2_d = nc.dram_tensor("ad2", [D, 1], f32, kind="ExternalInput")
    b1t_d = nc.dram_tensor("b1t", [P, D], f32, kind="ExternalInput")
    b2t_d = nc.dram_tensor("b2t", [P, D], f32, kind="ExternalInput")
    woutt_d = nc.dram_tensor("woutt", [1, D], f32, kind="ExternalInput")
    bout_d = nc.dram_tensor("bout", [1, 1], f32, kind="ExternalInput")
    out_d = nc.dram_tensor("out", [1, 1], f32, kind="ExternalOutput")

    # internal DRAM
    h1own = nc.dram_tensor("h1own", [cfg.npad, DT], bf16)
    tab1 = nc.dram_tensor("tab1", [cfg.ntab, DT], bf16, addr_space="Shared")
    out1 = nc.dram_tensor("out1", [cfg.npad, D], bf16)
    h2own = nc.dram_tensor("h2own", [cfg.npad, DT], bf16)
    tab2 = nc.dram_tensor("tab2", [cfg.ntab, DT], bf16, addr_space="Shared")
    gin = nc.dram_tensor("gin", [1, D], f32)
    gout = nc.dram_tensor("gout", [1, D], f32, addr_space="Shared")

    rg = [list(range(CORES))]
    AF = mybir.ActivationFunctionType
    OP = mybir.AluOpType

    with tile.TileContext(nc) as tc:
        with (
            tc.tile_pool(name="const", bufs=1) as cpool,
            tc.tile_pool(name="sbuf", bufs=2) as sb,
            tc.tile_pool(name="oh", bufs=4) as ohp,
            tc.tile_pool(name="psn", bufs=2, space="PSUM") as psn,
            tc.tile_pool(name="pse", bufs=2, space="PSUM") as pse,
            tc.tile_pool(name="psg", bufs=1, space="PSUM") as psg,
            tc.tile_pool(name="psw", bufs=1, space="PSUM") as psw,
        ):
            # ---- constants / one-time setup ----
            iota_t = cpool.tile([P, P], bf16)
            nc.gpsimd.iota(iota_t[:], pattern=[[1, P]], base=0,
                           channel_multiplier=0,
                           allow_small_or_imprecise_dtypes=True)
            ones_t = cpool.tile([P, 1], f32)
            nc.vector.memset(ones_t[:], 1.0)

            b1t_t = cpool.tile([P, D], f32)
            nc.sync.dma_start(out=b1t_t[:], in_=b1t_d[:, :])
            b2t_t = cpool.tile([P, D], f32)
            nc.sync.dma_start(out=b2t_t[:], in_=b2t_d[:, :])
            woutt_t = cpool.tile([1, D], f32)
            nc.sync.dma_start(out=woutt_t[:], in_=woutt_d[:, :])
            bout_t = cpool.tile([1, 1], f32)
            nc.sync.dma_start(out=bout_t[:], in_=bout_d[:, :])

            # edge index arrays, resident in SBUF
            esrc_t = cpool.tile([P, ctot], i32)
            nc.sync.dma_start(out=esrc_t[:], in_=esrc_d[:, :])
            eabs_t = cpool.tile([P, ctot], i32)
            nc.sync.dma_start(out=eabs_t[:], in_=eabs_d[:, :])
            erel_i = cpool.tile([P, ctot], i32)
            nc.sync.dma_start(out=erel_i[:], in_=erel_d[:, :])
            erel_f = cpool.tile([P, ctot], f32)
            nc.vector.tensor_copy(out=erel_f[:], in_=erel_i[:])

            # W_aug assembly, layer 1 (fp32)
            waug1 = cpool.tile([D, DT], f32)
            nc.sync.dma_start(out=waug1[:, 0:D], in_=w1_d[:, :])
            nc.vector.memset(waug1[:, D:DT], 0.0)
            w1t_t = cpool.tile([D, D], f32)
            nc.sync.dma_start(out=w1t_t[:], in_=w1t_d[:, :])
            av_t = cpool.tile([D, 2], f32)
            nc.sync.dma_start(out=av_t[:, 0:1], in_=as1_d[:, :])
            nc.sync.dma_start(out=av_t[:, 1:2], in_=ad1_d[:, :])
            pw = psw.tile([D, 2], f32, space="PSUM")
            nc.tensor.matmul(out=pw[:], lhsT=w1t_t[:], rhs=av_t[:],
                             start=True, stop=True)
            nc.vector.tensor_copy(out=waug1[:, D + 1:D + 3], in_=pw[:])

            # W_aug assembly, layer 2: build fp32, then split hi+lo bf16
            # (two accumulating bf16 matmuls recover fp32 weight precision)
            waug2f = cpool.tile([D, DT], f32)
            nc.sync.dma_start(out=waug2f[:, 0:D], in_=w2_d[:, :])
            nc.vector.memset(waug2f[:, D:DT], 0.0)
            w2t_t = cpool.tile([D, D], f32)
            nc.sync.dma_start(out=w2t_t[:], in_=w2t_d[:, :])
            av2_t = cpool.tile([D, 2], f32)
            nc.sync.dma_start(out=av2_t[:, 0:1], in_=as2_d[:, :])
            nc.sync.dma_start(out=av2_t[:, 1:2], in_=ad2_d[:, :])
            pw2 = psw.tile([D, 2], f32, space="PSUM")
            nc.tensor.matmul(out=pw2[:], lhsT=w2t_t[:], rhs=av2_t[:],
                             start=True, stop=True)
            nc.vector.tensor_copy(out=waug2f[:, D + 1:D + 3], in_=pw2[:])
            waug2_hi = cpool.tile([D, DT], bf16)
            nc.vector.tensor_copy(out=waug2_hi[:], in_=waug2f[:])
            waug2_lo = cpool.tile([D, DT], bf16)
            nc.vector.tensor_tensor(out=waug2_lo[:], in0=waug2f[:],
                                    in1=waug2_hi[:], op=OP.subtract)

            gsum = psg.tile([1, D], f32, space="PSUM")

            for layer in (1, 2):
                hown = h1own if layer == 1 else h2own
                tab = tab1 if layer == 1 else tab2
                btile = b1t_t if layer == 1 else b2t_t

                # ---- node phase: h_aug(own rows) = x @ W_aug ----
                for b0 in range(0, cfg.nt, tb_n):
                    b1 = min(b0 + tb_n, cfg.nt)
                    nb = b1 - b0
                    if layer == 1:
                        xt_t = sb.tile([D, tb_n * P], f32, tag="xt")
                        nc.sync.dma_start(
                            out=xt_t[:, 0:nb * P],
                            in_=xt_d[:, b0 * P:b1 * P])
                    else:
                        xt_t = sb.tile([D, tb_n * P], bf16, tag="xt2")
                        nc.sync.dma_start_transpose(
                            out=xt_t[:, 0:nb * P],
                            in_=out1[b0 * P:b1 * P, :])
                    stage_n = sb.tile([P, tb_n, DT], bf16, tag="stn")
                    for ti in range(nb):
                        pn = psn.tile([P, DT], f32, space="PSUM")
                        if layer == 1:
                            nc.tensor.matmul(
                                out=pn[:], lhsT=xt_t[:, ti * P:(ti + 1) * P],
                                rhs=waug1[:], start=True, stop=True)
                        else:
                            nc.tensor.matmul(
                                out=pn[:], lhsT=xt_t[:, ti * P:(ti + 1) * P],
                                rhs=waug2_hi[:], start=True, stop=False)
                            nc.tensor.matmul(
                                out=pn[:], lhsT=xt_t[:, ti * P:(ti + 1) * P],
                                rhs=waug2_lo[:], start=False, stop=True)
                        nc.vector.tensor_copy(out=stage_n[:, ti, :], in_=pn[:])
                        nc.vector.memset(stage_n[:, ti, D:D + 1], 1.0)
                    nc.sync.dma_start(
                        out=hown[b0 * P:b1 * P, :].rearrange(
                            "(k p) d -> p k d", p=P),
                        in_=stage_n[:, 0:nb, :])

                # ---- AllGather the node table ----
                nc.gpsimd.collective_compute(
                    "AllGather", OP.bypass, replica_groups=rg,
                    ins=[hown[:, :]], outs=[tab[:, :]])

                # ---- edge phase ----
                for b0 in range(0, cfg.nt, tb_e):
                    b1 = min(b0 + tb_e, cfg.nt)
                    nb = b1 - b0
                    cb = nb * c_fix
                    cs = slice(b0 * c_fix, b1 * c_fix)
                    # walrus only implements indirect DMA with ONE index per
                    # partition -> one [128,1] gather per 128-edge chunk
                    g_t = sb.tile([P, tb_e * c_fix, DT], bf16, tag="g")
                    sd_t = sb.tile([P, tb_e * c_fix, 1], bf16, tag="sd")
                    for cloc in range(cb):
                        cg0 = b0 * c_fix + cloc
                        nc.gpsimd.indirect_dma_start(
                            out=g_t[:, cloc, :], out_offset=None,
                            in_=tab[:, :],
                            in_offset=bass.IndirectOffsetOnAxis(
                                ap=esrc_t[:, cg0:cg0 + 1], axis=0))
                        nc.gpsimd.indirect_dma_start(
                            out=sd_t[:, cloc, :], out_offset=None,
                            in_=tab[:, :],
                            in_offset=bass.IndirectOffsetOnAxis(
                                ap=eabs_t[:, cg0:cg0 + 1], axis=0),
                            element_offset=D + 2)
                    z_t = sb.tile([P, tb_e * c_fix], f32, tag="z")
                    nc.vector.tensor_tensor(
                        out=z_t[:, 0:cb], in0=g_t[:, 0:cb, D + 1],
                        in1=sd_t[:, 0:cb, 0], op=OP.add)
                    zs_t = sb.tile([P, tb_e * c_fix], f32, tag="zs")
                    nc.vector.tensor_scalar(
                        out=zs_t[:, 0:cb], in0=z_t[:, 0:cb], scalar1=NEG,
                        scalar2=None, op0=OP.mult)
                    lr_t = sb.tile([P, tb_e * c_fix], f32, tag="lr")
                    nc.vector.tensor_tensor(
                        out=lr_t[:, 0:cb], in0=z_t[:, 0:cb],
                        in1=zs_t[:, 0:cb], op=OP.max)
                    w_t = sb.tile([P, tb_e * c_fix], f32, tag="w")
                    nc.scalar.activation(out=w_t[:, 0:cb], in_=lr_t[:, 0:cb],
                                         func=AF.Exp)

                    if layer == 1:
                        stage_o = sb.tile([P, tb_e, D], bf16, tag="sto")
                    for ti in range(nb):
                        pe_t = pse.tile([P, D + 1], f32, space="PSUM")
                        for c in range(c_fix):
                            cg = ti * c_fix + c
                            oh = ohp.tile([P, P], bf16, tag="oh")
                            nc.vector.tensor_scalar(
                                out=oh[:], in0=iota_t[:],
                                scalar1=erel_f[:, (b0 + ti) * c_fix + c:
                                               (b0 + ti) * c_fix + c + 1],
                                scalar2=w_t[:, cg:cg + 1],
                                op0=OP.is_equal, op1=OP.mult)
                            nc.tensor.matmul(
                                out=pe_t[:], lhsT=oh[:],
                                rhs=g_t[:, cg, 0:D + 1],
                                start=(c == 0), stop=(c == c_fix - 1))
                        # finalize tile
                        gt = b0 + ti
                        rows = cfg.last_rows if gt == cfg.nt - 1 else P
                        dplus = sb.tile([P, 1], f32, tag="dp")
                        nc.vector.tensor_scalar(
                            out=dplus[:], in0=pe_t[:, D:D + 1], scalar1=EPS,
                            scalar2=None, op0=OP.add)
                        recip = sb.tile([P, 1], f32, tag="rc")
                        nc.vector.reciprocal(out=recip[:], in_=dplus[:])
                        o1 = sb.tile([P, D], f32, tag="o1")
                        nc.vector.tensor_scalar(
                            out=o1[:], in0=pe_t[:, 0:D], scalar1=recip[:],
                            scalar2=None, op0=OP.mult)
                        o2 = sb.tile([P, D], f32, tag="o2")
                        nc.vector.tensor_tensor(
                            out=o2[:], in0=o1[:], in1=btile[:], op=OP.add)
                        if layer == 1:
                            nc.scalar.activation(
                                out=stage_o[:, ti, :], in_=o2[:],
                                func=AF.Relu)
                        else:
                            o3 = sb.tile([P, D], f32, tag="o3")
                            nc.scalar.activation(out=o3[:], in_=o2[:],
                                                 func=AF.Relu)
                            nc.tensor.matmul(
                                out=gsum[:], lhsT=ones_t[0:rows, :],
                                rhs=o3[0:rows, :],
                                start=(gt == 0), stop=(gt == cfg.nt - 1),
                                skip_group_check=True)
                    if layer == 1:
                        nc.sync.dma_start(
                            out=out1[b0 * P:b1 * P, :].rearrange(
                                "(k p) d -> p k d", p=P),
                            in_=stage_o[:, 0:nb, :])

            # ---- head: mean pool + linear ----
            g_sb = sb.tile([1, D], f32, tag="gsb")
            nc.vector.tensor_copy(out=g_sb[:], in_=gsum[:])
            nc.sync.dma_start(out=gin[:, :], in_=g_sb[:])
            nc.gpsimd.collective_compute(
                "AllReduce", OP.add, replica_groups=rg,
                ins=[gin[:, :]], outs=[gout[:, :]])
            g2_sb = sb.tile([1, D], f32, tag="g2sb")
            nc.sync.dma_start(out=g2_sb[:], in_=gout[:, :])
            junk = sb.tile([1, D], f32, tag="junk")
            res = sb.tile([1, 1], f32, tag="res")
            nc.vector.scalar_tensor_tensor(
                out=junk[:], in0=g2_sb[:], scalar=1.0 / cfg.n_nodes,
                in1=woutt_t[:], op0=OP.mult, op1=OP.mult,
                accum_out=res[:])
            res2 = sb.tile([1, 1], f32, tag="res2")
            nc.vector.tensor_tensor(out=res2[:], in0=res[:], in1=bout_t[:],
                                    op=OP.add)
            nc.sync.dma_start(out=out_d[:, :], in_=res2[:])

            if debug_dumps:
                for nm, src, shp, dt_ in [
                    ("d_h1", h1own, [cfg.npad, DT], bf16),
                    ("d_tab1", tab1, [cfg.ntab, DT], bf16),
                    ("d_out1", out1, [cfg.npad, D], bf16),
                    ("d_h2", h2own, [cfg.npad, DT], bf16),
                    ("d_gin", gin, [1, D], f32),
                    ("d_gout", gout, [1, D], f32),
                ]:
                    dd = nc.dram_tensor(nm, shp, dt_, kind="ExternalOutput")
                    nc.sync.dma_start(out=dd[:, :], in_=src[:, :])

    nc.compile()
    return nc


def kernel(**inputs):
    cfg = Cfg(N_NODES)
    in_maps, c_fix = prep_inputs(inputs, cfg)
    nc = build_kernel(cfg, c_fix)
    from concourse.bass_utils import run_bass_kernel_spmd
    res = run_bass_kernel_spmd(nc, in_maps, list(range(CORES)))
    return np.asarray(res.results[0]["out"], np.float32)


# revision 21
# speedup vs baseline: 1.6344x; 1.6344x over previous
"""GAT regression kernel for Trainium2, distributed over 8 NeuronCores.

Strategy (dst-node sharding, edge lists sorted by destination):
  - Each core owns a contiguous range of destination nodes (N/8).
  - Host sorts edges (incl. self-loops) by dst, shards them by dst range,
    groups them into 128-node dst tiles, and packs them into fixed-size
    chunk slots (128 edges per chunk, C_FIX chunks per tile).
  - Per layer: each core computes the node transform h_aug = x @ W_aug for
    its own nodes ([h | 0 | s_src | s_dst] + a const-one column), then an
    AllGather replicates the full bf16 node table to every core.
  - Edge phase: batched indirect-DMA gathers pull h_aug rows by src id and
    s_dst by dst id; attention weights w = exp(max(z, 0.2 z)) are computed
    in bulk; a per-chunk "scaled one-hot" (w_e at column dst_e) feeds a
    PE matmul that segment-sums w_e * [h_src | 1] into PSUM per dst tile.
  - Layer-2 outputs are column-summed via ones-matmuls, AllReduced, and the
    final linear head produces the [1,1] output on every core.
"""

import sys

for _p in ("/opt/trn_rl_repo", "/opt/trn_rl_repo/concourse"):
    if _p not in sys.path:
        sys.path.insert(0, _p)

import numpy as np
import ml_dtypes

import concourse.bass as bass
import concourse.mybir as mybir
import concourse.tile as tile
from concourse import bacc

P = 128
D = 64            # feature dim (both layers)
DT = 68           # table row: [h(0:64), one(64), s_src(65), s_dst(66), pad]
CORES = 8
NEG = 0.2         # leaky relu slope
EPS = 1e-16

# problem sizes (hardcoded per spec)
N_NODES = 100000
E_RAW = 1200000

f32 = mybir.dt.float32
bf16 = mybir.dt.bfloat16
i32 = mybir.dt.int32
bf16_np = ml_dtypes.bfloat16


class Cfg:
    def __init__(self, n_nodes):
        assert n_nodes % CORES == 0
        self.n_nodes = n_nodes
        self.nown = n_nodes // CORES          # real nodes per core
        self.nt = -(-self.nown // P)          # dst tiles per core
        self.npad = self.nt * P               # padded nodes per core
        self.ntab = CORES * self.npad         # padded global table rows
        self.last_rows = self.nown - (self.nt - 1) * P


def _pad_gid(node_ids, cfg):
    """global node id -> row in the padded AllGather table"""
    return (node_ids // cfg.nown) * cfg.npad + (node_ids % cfg.nown)


def prep_inputs(inputs, cfg):
    """Host-side sharding: index manipulation + dtype/layout prep only."""
    x = np.asarray(inputs["x"], np.float32)
    ei = np.asarray(inputs["edge_index"], np.int64)
    loop = np.arange(cfg.n_nodes, dtype=np.int64)
    src = np.concatenate([ei[0], loop])
    dst = np.concatenate([ei[1], loop])
    order = np.argsort(dst, kind="stable")
    src_s, dst_s = src[order], dst[order]

    core_bounds = np.searchsorted(dst_s, np.arange(CORES + 1) * cfg.nown)
    # first pass: global max chunks per tile
    c_fix = 1
    per_core = []
    for k in range(CORES):
        sl = slice(core_bounds[k], core_bounds[k + 1])
        dloc = dst_s[sl] - k * cfg.nown
        sglo = src_s[sl]
        tb = np.searchsorted(dloc, np.arange(cfg.nt + 1) * P)
        cnts = np.diff(tb)
        c_fix = max(c_fix, int(-(-cnts.max() // P)))
        per_core.append((dloc, sglo, tb))

    ctot = cfg.nt * c_fix
    in_maps = []
    w1 = np.asarray(inputs["W1"], np.float32)
    w2 = np.asarray(inputs["W2"], np.float32)
    shared = {
        "w1": w1,
        "w1t": np.ascontiguousarray(w1.T),
        "as1": np.asarray(inputs["a_src1"], np.float32).reshape(D, 1),
        "ad1": np.asarray(inputs["a_dst1"], np.float32).reshape(D, 1),
        "w2": w2,
        "w2t": np.ascontiguousarray(w2.T),
        "as2": np.asarray(inputs["a_src2"], np.float32).reshape(D, 1),
        "ad2": np.asarray(inputs["a_dst2"], np.float32).reshape(D, 1),
        "b1t": np.ascontiguousarray(
            np.broadcast_to(np.asarray(inputs["b1"], np.float32), (P, D))),
        "b2t": np.ascontiguousarray(
            np.broadcast_to(np.asarray(inputs["b2"], np.float32), (P, D))),
        "woutt": np.asarray(inputs["W_out"], np.float32).reshape(1, D),
        "bout": np.asarray(inputs["b_out"], np.float32).reshape(1, 1),
    }
    for k in range(CORES):
        dloc, sglo, tb = per_core[k]
        src_arr = np.zeros((P, ctot), np.int32)
        rel_arr = np.full((P, ctot), -1, np.int32)
        abs_arr = np.zeros((P, ctot), np.int32)
        for t in range(cfg.nt):
            e0, e1 = tb[t], tb[t + 1]
            n = e1 - e0
            if n == 0:
                continue
            nslots = c_fix * P
            s_pad = np.zeros(nslots, np.int64)
            r_pad = np.full(nslots, -1, np.int64)
            a_pad = np.zeros(nslots, np.int64)
            s_pad[:n] = _pad_gid(sglo[e0:e1], cfg)
            r_pad[:n] = dloc[e0:e1] - t * P
            a_pad[:n] = _pad_gid(dloc[e0:e1] + k * cfg.nown, cfg)
            cs = slice(t * c_fix, (t + 1) * c_fix)
            src_arr[:, cs] = s_pad.reshape(c_fix, P).T
            rel_arr[:, cs] = r_pad.reshape(c_fix, P).T
            abs_arr[:, cs] = a_pad.reshape(c_fix, P).T
        xt = np.zeros((D, cfg.npad), np.float32)
        xt[:, :cfg.nown] = x[k * cfg.nown:(k + 1) * cfg.nown].T
        in_maps.append({
            "xt": xt, "esrc": src_arr, "erel": rel_arr, "eabs": abs_arr,
            "erelrow": np.ascontiguousarray(rel_arr.T).astype(bf16_np),
            **shared,
        })
    return in_maps, c_fix


def build_kernel(cfg, c_fix, tb_n=8, tb_e=8, debug_dumps=False):
    """Build the SPMD Bass program (same program for all 8 cores)."""
    ctot = cfg.nt * c_fix
    nc = bacc.Bacc("TRN2", target_bir_lowering=False, debug=False,
                   num_devices=CORES)

    # I/O
    xt_d = nc.dram_tensor("xt", [D, cfg.npad], f32, kind="ExternalInput")
    esrc_d = nc.dram_tensor("esrc", [P, ctot], i32, kind="ExternalInput")
    erel_d = nc.dram_tensor("erel", [P, ctot], i32, kind="ExternalInput")
    eabs_d = nc.dram_tensor("eabs", [P, ctot], i32, kind="ExternalInput")
    erelrow_d = nc.dram_tensor("erelrow", [ctot, P], bf16, kind="ExternalInput")
    w1_d = nc.dram_tensor("w1", [D, D], f32, kind="ExternalInput")
    w1t_d = nc.dram_tensor("w1t", [D, D], f32, kind="ExternalInput")
    as1_d = nc.dram_tensor("as1", [D, 1], f32, kind="ExternalInput")
    ad1_d = nc.dram_tensor("ad1", [D, 1], f32, kind="ExternalInput")
    w2_d = nc.dram_tensor("w2", [D, D], f32, kind="ExternalInput")
    w2t_d = nc.dram_tensor("w2t", [D, D], f32, kind="ExternalInput")
    as2_d = nc.dram_tensor("as2", [D, 1], f32, kind="ExternalInput")
    ad2_d = nc.dram_tensor("ad2", [D, 1], f32, kind="ExternalInput")
    b1t_d = nc.dram_tensor("b1t", [P, D], f32, kind="ExternalInput")
    b2t_d = nc.dram_tensor("b2t", [P, D], f32, kind="ExternalInput")
    woutt_d = nc.dram_tensor("woutt", [1, D], f32, kind="ExternalInput")
    bout_d = nc.dram_tensor("bout", [1, 1], f32, kind="ExternalInput")
    out_d = nc.dram_tensor("out", [1, 1], f32, kind="ExternalOutput")

    # internal DRAM
    h1own = nc.dram_tensor("h1own", [cfg.npad, DT], bf16)
    tab1 = nc.dram_tensor("tab1", [cfg.ntab, DT], bf16, addr_space="Shared")
    out1 = nc.dram_tensor("out1", [cfg.npad, D], bf16)
    h2own = nc.dram_tensor("h2own", [cfg.npad, DT], bf16)
    tab2 = nc.dram_tensor("tab2", [cfg.ntab, DT], bf16, addr_space="Shared")
    gin = nc.dram_tensor("gin", [1, D], f32)
    gout = nc.dram_tensor("gout", [1, D], f32, addr_space="Shared")

    rg = [list(range(CORES))]
    AF = mybir.ActivationFunctionType
    OP = mybir.AluOpType

    with tile.TileContext(nc) as tc:
        with (
            tc.tile_pool(name="const", bufs=1) as cpool,
            tc.tile_pool(name="sbuf", bufs=2) as sb,
            tc.tile_pool(name="oh", bufs=4) as ohp,
            tc.tile_pool(name="psn", bufs=1, space="PSUM") as psn,
            tc.tile_pool(name="pse", bufs=2, space="PSUM") as pse,
            tc.tile_pool(name="psg", bufs=1, space="PSUM") as psg,
            tc.tile_pool(name="psw", bufs=1, space="PSUM") as psw,
            tc.tile_pool(name="psr", bufs=1, space="PSUM") as psr,
            tc.tile_pool(name="psd", bufs=2, space="PSUM") as psd,
        ):
            # ---- constants / one-time setup ----
            iota_t = cpool.tile([P, P], bf16)
            nc.gpsimd.iota(iota_t[:], pattern=[[1, P]], base=0,
                           channel_multiplier=0,
                           allow_small_or_imprecise_dtypes=True)
            ones_t = cpool.tile([P, 1], f32)
            nc.vector.memset(ones_t[:], 1.0)
            ones1_bf = cpool.tile([1, P], bf16)
            nc.vector.memset(ones1_bf[:], 1.0)
            iota_col = cpool.tile([P, 1], f32)
            nc.gpsimd.iota(iota_col[:], pattern=[[1, 1]], base=0,
                           channel_multiplier=1,
                           allow_small_or_imprecise_dtypes=True)

            b1t_t = cpool.tile([P, D], f32)
            nc.sync.dma_start(out=b1t_t[:], in_=b1t_d[:, :])
            b2t_t = cpool.tile([P, D], f32)
            nc.sync.dma_start(out=b2t_t[:], in_=b2t_d[:, :])
            woutt_t = cpool.tile([1, D], f32)
            nc.sync.dma_start(out=woutt_t[:], in_=woutt_d[:, :])
            bout_t = cpool.tile([1, 1], f32)
            nc.sync.dma_start(out=bout_t[:], in_=bout_d[:, :])

            # edge index arrays, resident in SBUF
            esrc_t = cpool.tile([P, ctot], i32)
            nc.sync.dma_start(out=esrc_t[:], in_=esrc_d[:, :])
            erel_i = cpool.tile([P, ctot], i32)
            nc.sync.dma_start(out=erel_i[:], in_=erel_d[:, :])
            erel_f = cpool.tile([P, ctot], f32)
            nc.vector.tensor_copy(out=erel_f[:], in_=erel_i[:])

            # W_aug assembly, layer 1 (fp32)
            waug1 = cpool.tile([D, DT], f32)
            nc.sync.dma_start(out=waug1[:, 0:D], in_=w1_d[:, :])
            nc.vector.memset(waug1[:, D:DT], 0.0)
            w1t_t = cpool.tile([D, D], f32)
            nc.sync.dma_start(out=w1t_t[:], in_=w1t_d[:, :])
            av_t = cpool.tile([D, 2], f32)
            nc.sync.dma_start(out=av_t[:, 0:1], in_=as1_d[:, :])
            nc.sync.dma_start(out=av_t[:, 1:2], in_=ad1_d[:, :])
            pw = psw.tile([D, 2], f32, space="PSUM")
            nc.tensor.matmul(out=pw[:], lhsT=w1t_t[:], rhs=av_t[:],
                             start=True, stop=True)
            nc.vector.tensor_copy(out=waug1[:, D + 1:D + 3], in_=pw[:])

            # W_aug assembly, layer 2: build fp32, then split hi+lo bf16
            # (two accumulating bf16 matmuls recover fp32 weight precision)
            waug2f = cpool.tile([D, DT], f32)
            nc.sync.dma_start(out=waug2f[:, 0:D], in_=w2_d[:, :])
            nc.vector.memset(waug2f[:, D:DT], 0.0)
            w2t_t = cpool.tile([D, D], f32)
            nc.sync.dma_start(out=w2t_t[:], in_=w2t_d[:, :])
            av2_t = cpool.tile([D, 2], f32)
            nc.sync.dma_start(out=av2_t[:, 0:1], in_=as2_d[:, :])
            nc.sync.dma_start(out=av2_t[:, 1:2], in_=ad2_d[:, :])
            pw2 = psw.tile([D, 2], f32, space="PSUM", tag="pw")
            nc.tensor.matmul(out=pw2[:], lhsT=w2t_t[:], rhs=av2_t[:],
                             start=True, stop=True)
            nc.vector.tensor_copy(out=waug2f[:, D + 1:D + 3], in_=pw2[:])
            waug2_hi = cpool.tile([D, DT], bf16)
            nc.vector.tensor_copy(out=waug2_hi[:], in_=waug2f[:])
            waug2_lo = cpool.tile([D, DT], bf16)
            nc.vector.tensor_tensor(out=waug2_lo[:], in0=waug2f[:],
                                    in1=waug2_hi[:], op=OP.subtract)

            gsum = psg.tile([1, D], f32, space="PSUM")

            for layer in (1, 2):
                hown = h1own if layer == 1 else h2own
                tab = tab1 if layer == 1 else tab2
                btile = b1t_t if layer == 1 else b2t_t

                # ---- node phase: h_aug(own rows) = x @ W_aug ----
                for b0 in range(0, cfg.nt, tb_n):
                    b1 = min(b0 + tb_n, cfg.nt)
                    nb = b1 - b0
                    if layer == 1:
                        xt_t = sb.tile([D, tb_n * P], f32, tag="xt")
                        nc.sync.dma_start(
                            out=xt_t[:, 0:nb * P],
                            in_=xt_d[:, b0 * P:b1 * P])
                    else:
                        xt_t = sb.tile([D, tb_n * P], bf16, tag="xt2")
                        nc.sync.dma_start_transpose(
                            out=xt_t[:, 0:nb * P],
                            in_=out1[b0 * P:b1 * P, :])
                    stage_n = sb.tile([P, tb_n, DT], bf16, tag="stn")
                    for ti in range(nb):
                        pn = psn.tile([P, DT], f32, space="PSUM")
                        if layer == 1:
                            nc.tensor.matmul(
                                out=pn[:], lhsT=xt_t[:, ti * P:(ti + 1) * P],
                                rhs=waug1[:], start=True, stop=True)
                        else:
                            nc.tensor.matmul(
                                out=pn[:], lhsT=xt_t[:, ti * P:(ti + 1) * P],
                                rhs=waug2_hi[:], start=True, stop=False)
                            nc.tensor.matmul(
                                out=pn[:], lhsT=xt_t[:, ti * P:(ti + 1) * P],
                                rhs=waug2_lo[:], start=False, stop=True)
                        nc.vector.tensor_copy(out=stage_n[:, ti, :], in_=pn[:])
                        nc.vector.memset(stage_n[:, ti, D:D + 1], 1.0)
                    nc.sync.dma_start(
                        out=hown[b0 * P:b1 * P, :].rearrange(
                            "(k p) d -> p k d", p=P),
                        in_=stage_n[:, 0:nb, :])

                # ---- AllGather the node table ----
                nc.gpsimd.collective_compute(
                    "AllGather", OP.bypass, replica_groups=rg,
                    ins=[hown[:, :]], outs=[tab[:, :]])

                # ---- edge phase ----
                for b0 in range(0, cfg.nt, tb_e):
                    b1 = min(b0 + tb_e, cfg.nt)
                    nb = b1 - b0
                    cb = nb * c_fix
                    cs = slice(b0 * c_fix, b1 * c_fix)
                    # walrus only implements indirect DMA with ONE index per
                    # partition -> one [128,1] gather per 128-edge chunk
                    g_t = sb.tile([P, tb_e * c_fix, DT], bf16, tag="g")
                    for cloc in range(cb):
                        cg0 = b0 * c_fix + cloc
                        nc.gpsimd.indirect_dma_start(
                            out=g_t[:, cloc, :], out_offset=None,
                            in_=tab[:, :],
                            in_offset=bass.IndirectOffsetOnAxis(
                                ap=esrc_t[:, cg0:cg0 + 1], axis=0))
                    # s_dst per edge via transposed one-hot matmuls instead
                    # of a second per-edge gather: replicate the chunk's
                    # dstrel row across partitions (K=1 ones-matmul), build
                    # onehotT = (dstrel_row == partition), then matmul with
                    # the tile's contiguous s_dst column.
                    erow_t = sb.tile([1, tb_e * c_fix * P], bf16, tag="erow")
                    nc.sync.dma_start(
                        out=erow_t[:, 0:cb * P],
                        in_=erelrow_d[b0 * c_fix:b1 * c_fix, :].rearrange(
                            "c p -> (c p)").unsqueeze(0))
                    sdcol_t = sb.tile([P, tb_e, 1], bf16, tag="sdcol")
                    nc.sync.dma_start(
                        out=sdcol_t[:, 0:nb, :],
                        in_=hown[b0 * P:b1 * P, D + 2:D + 3].rearrange(
                            "(k p) d -> p k d", p=P))
                    psd_b = psd.tile([P, tb_e * c_fix], f32, space="PSUM")
                    for cloc in range(cb):
                        prepl = psr.tile([P, P], f32, space="PSUM")
                        nc.tensor.matmul(
                            out=prepl[:], lhsT=ones1_bf[:],
                            rhs=erow_t[:, cloc * P:(cloc + 1) * P],
                            start=True, stop=True)
                        ohT = ohp.tile([P, P], bf16, tag="ohT")
                        nc.vector.tensor_scalar(
                            out=ohT[:], in0=prepl[:], scalar1=iota_col[:],
                            scalar2=None, op0=OP.is_equal)
                        nc.tensor.matmul(
                            out=psd_b[:, cloc:cloc + 1], lhsT=ohT[:],
                            rhs=sdcol_t[:, cloc // c_fix, :],
                            start=True, stop=True, skip_group_check=True)
                    z_t = sb.tile([P, tb_e * c_fix], f32, tag="z")
                    nc.vector.tensor_tensor(
                        out=z_t[:, 0:cb], in0=g_t[:, 0:cb, D + 1],
                        in1=psd_b[:, 0:cb], op=OP.add)
                    zs_t = sb.tile([P, tb_e * c_fix], f32, tag="zs")
                    nc.vector.tensor_scalar(
                        out=zs_t[:, 0:cb], in0=z_t[:, 0:cb], scalar1=NEG,
                        scalar2=None, op0=OP.mult)
                    lr_t = sb.tile([P, tb_e * c_fix], f32, tag="lr")
                    nc.vector.tensor_tensor(
                        out=lr_t[:, 0:cb], in0=z_t[:, 0:cb],
                        in1=zs_t[:, 0:cb], op=OP.max)
                    w_t = sb.tile([P, tb_e * c_fix], f32, tag="w")
                    nc.scalar.activation(out=w_t[:, 0:cb], in_=lr_t[:, 0:cb],
                                         func=AF.Exp)

                    if layer == 1:
                        stage_o = sb.tile([P, tb_e, D], bf16, tag="sto")
                    for ti in range(nb):
                        pe_t = pse.tile([P, D + 1], f32, space="PSUM")
                        for c in range(c_fix):
                            cg = ti * c_fix + c
                            oh = ohp.tile([P, P], bf16, tag="oh")
                            nc.vector.tensor_scalar(
                                out=oh[:], in0=iota_t[:],
                                scalar1=erel_f[:, (b0 + ti) * c_fix + c:
                                               (b0 + ti) * c_fix + c + 1],
                                scalar2=w_t[:, cg:cg + 1],
                                op0=OP.is_equal, op1=OP.mult)
                            nc.tensor.matmul(
                                out=pe_t[:], lhsT=oh[:],
                                rhs=g_t[:, cg, 0:D + 1],
                                start=(c == 0), stop=(c == c_fix - 1))
                        # finalize tile
                        gt = b0 + ti
                        rows = cfg.last_rows if gt == cfg.nt - 1 else P
                        dplus = sb.tile([P, 1], f32, tag="dp")
                        nc.vector.tensor_scalar(
                            out=dplus[:], in0=pe_t[:, D:D + 1], scalar1=EPS,
                            scalar2=None, op0=OP.add)
                        recip = sb.tile([P, 1], f32, tag="rc")
                        nc.vector.reciprocal(out=recip[:], in_=dplus[:])
                        o1 = sb.tile([P, D], f32, tag="o1")
                        nc.vector.tensor_scalar(
                            out=o1[:], in0=pe_t[:, 0:D], scalar1=recip[:],
                            scalar2=None, op0=OP.mult)
                        o2 = sb.tile([P, D], f32, tag="o2")
                        nc.vector.tensor_tensor(
                            out=o2[:], in0=o1[:], in1=btile[:], op=OP.add)
                        if layer == 1:
                            nc.scalar.activation(
                                out=stage_o[:, ti, :], in_=o2[:],
                                func=AF.Relu)
                        else:
                            o3 = sb.tile([P, D], f32, tag="o3")
                            nc.scalar.activation(out=o3[:], in_=o2[:],
                                                 func=AF.Relu)
                            nc.tensor.matmul(
                                out=gsum[:], lhsT=ones_t[0:rows, :],
                                rhs=o3[0:rows, :],
                                start=(gt == 0), stop=(gt == cfg.nt - 1),
                                skip_group_check=True)
                    if layer == 1:
                        nc.sync.dma_start(
                            out=out1[b0 * P:b1 * P, :].rearrange(
                                "(k p) d -> p k d", p=P),
                            in_=stage_o[:, 0:nb, :])

            # ---- head: mean pool + linear ----
            g_sb = sb.tile([1, D], f32, tag="gsb")
            nc.vector.tensor_copy(out=g_sb[:], in_=gsum[:])
            nc.sync.dma_start(out=gin[:, :], in_=g_sb[:])
            nc.gpsimd.collective_compute(
                "AllReduce", OP.add, replica_groups=rg,
                ins=[gin[:, :]], outs=[gout[:, :]])
            g2_sb = sb.tile([1, D], f32, tag="g2sb")
            nc.sync.dma_start(out=g2_sb[:], in_=gout[:, :])
            junk = sb.tile([1, D], f32, tag="junk")
            res = sb.tile([1, 1], f32, tag="res")
            nc.vector.scalar_tensor_tensor(
                out=junk[:], in0=g2_sb[:], scalar=1.0 / cfg.n_nodes,
                in1=woutt_t[:], op0=OP.mult, op1=OP.mult,
                accum_out=res[:])
            res2 = sb.tile([1, 1], f32, tag="res2")
            nc.vector.tensor_tensor(out=res2[:], in0=res[:], in1=bout_t[:],
                                    op=OP.add)
            nc.sync.dma_start(out=out_d[:, :], in_=res2[:])

            if debug_dumps:
                for nm, src, shp, dt_ in [
                    ("d_h1", h1own, [cfg.npad, DT], bf16),
                    ("d_tab1", tab1, [cfg.ntab, DT], bf16),
                    ("d_out1", out1, [cfg.npad, D], bf16),
                    ("d_h2", h2own, [cfg.npad, DT], bf16),
                    ("d_gin", gin, [1, D], f32),
                    ("d_gout", gout, [1, D], f32),
                ]:
                    dd = nc.dram_tensor(nm, shp, dt_, kind="ExternalOutput")
                    nc.sync.dma_start(out=dd[:, :], in_=src[:, :])

    nc.compile()
    return nc


def kernel(**inputs):
    cfg = Cfg(N_NODES)
    in_maps, c_fix = prep_inputs(inputs, cfg)
    nc = build_kernel(cfg, c_fix)
    from concourse.bass_utils import run_bass_kernel_spmd
    res = run_bass_kernel_spmd(nc, in_maps, list(range(CORES)))
    return np.asarray(res.results[0]["out"], np.float32)
